# revision 1
# baseline (speedup 1.0000x reference)
"""Trainium2 Bass kernel for causal multi-head self-attention.

nn.Module: y = MHSA(x) with D=768, H=12 heads, d_k=64, S=4096, causal mask,
torch-Linear convention (y = x @ W.T, no bias).

Distribution over the 8 NeuronCores (no collectives — host-side gather
between two device launches):

  Launch 1 (SPMD, all cores run the same program): QKV projections,
  sequence-sharded. Core c projects x rows [512c, 512c+512) against all of
  W_q/W_k/W_v, emitting Q^T and K^T (head-dim-major) and V (natural), all
  bf16. The host concatenates the shards (pure gather).

  Launch 2 (MPMD, one program variant per core): attention + W_o,
  query-sharded with zig-zag causal load balancing: core c owns the two
  256-row query blocks (c, 15-c), so every core does an equal amount of
  causal work. Scores are computed transposed (scores^T[kv, q], K-tile
  stationary / Q^T moving) so the softmax numerator feeds the P^T @ V
  matmul with no PE transposes. Softmax skips max-subtraction (scores are
  ~N(0,1); exp cannot overflow in fp32) and gets denominators for free via
  a ones-column appended to V. The strict-upper causal mask is applied
  additively (-1e9) to the two diagonal kv-tiles per block only. The
  per-column denominators are broadcast across partitions with a tiny
  selector matmul on the PE, and the normalized attention output (already
  transposed = exactly the W_o contraction layout) goes through W_o.
  Core c returns y rows for its two blocks; the host scatters them back.

All matmuls are bf16 with fp32 PSUM accumulation.
"""

import numpy as np
import ml_dtypes
import jax

import concourse.bass as bass
import concourse.tile as tile
import concourse.mybir as mybir
from concourse import bacc, bass2jax

BF16 = mybir.dt.bfloat16
F32 = mybir.dt.float32
F32R = mybir.dt.float32r
AF = mybir.ActivationFunctionType

B = 1
D = 768          # d_model
S = 4096         # sequence length
H = 12           # heads
DK = 64          # head dim
NC = 8           # NeuronCores
NB = 16          # 256-row query blocks
QB = S // NB     # 256
SC = S // NC     # 512 rows per core
NT = D // 128    # 6
NEG = -1e9

_bf = lambda a: np.ascontiguousarray(a).astype(ml_dtypes.bfloat16)


def _blocks_for_core(c):
    return (c, NB - 1 - c)


# --------------------------------------------------------------------------
# MPMD runner: run a (possibly different) bass program on each NeuronCore
# concurrently via the bass_exec custom-call machinery.
# --------------------------------------------------------------------------

def _io_names(nc):
    in_names, out_names, out_avals = [], [], []
    pname = nc.partition_id_tensor.name if nc.partition_id_tensor else None
    for alloc in nc.m.functions[0].allocations:
        if not isinstance(alloc, mybir.MemoryLocationSet):
            continue
        name = alloc.memorylocations[0].name
        if alloc.kind == "ExternalInput":
            if name != pname:
                in_names.append(name)
        elif alloc.kind == "ExternalOutput":
            out_names.append(name)
            out_avals.append(
                jax.core.ShapedArray(
                    tuple(alloc.tensor_shape), mybir.dt.np(alloc.dtype)))
    return in_names, out_names, out_avals, pname


_jit_cache = {}


def run_mpmd(ncs, in_maps):
    """ncs: one compiled Bacc program per core (entries may repeat);
    in_maps: per-core dict name->np.ndarray. Returns per-core output dicts."""
    bass2jax.install_neuronx_cc_hook()
    devices = jax.devices()[: len(ncs)]
    futs, metas = [], []
    for core_id, (nc, in_map, dev) in enumerate(
            zip(ncs, in_maps, devices, strict=True)):
        in_names, out_names, out_avals, pname = _io_names(nc)
        key = (id(nc), core_id)
        if key not in _jit_cache:
            all_names = tuple(in_names + out_names + ([pname] if pname else []))

            def _body(*args, _nc=nc, _avals=tuple(out_avals),
                      _names=all_names, _onames=tuple(out_names)):
                return tuple(bass2jax._bass_exec_p.bind(
                    *args, out_avals=_avals, in_names=_names,
                    out_names=_onames, lowering_input_output_aliases=(),
                    sim_require_finite=True, sim_require_nnan=True, nc=_nc))

            n_params = len(in_names)
            donate = tuple(range(n_params, n_params + len(out_avals)))
            _jit_cache[key] = jax.jit(
                _body, donate_argnums=donate, keep_unused=True)
        fn = _jit_cache[key]
        dev_args = [jax.device_put(np.asarray(in_map[n]), dev)
                    for n in in_names]
        dev_zeros = [jax.device_put(np.zeros(a.shape, a.dtype), dev)
                     for a in out_avals]
        extra = ([jax.device_put(np.array([[core_id]], np.uint32), dev)]
                 if pname else [])
        futs.append(fn(*dev_args, *dev_zeros, *extra))
        metas.append(out_names)
    return [
        {n: np.asarray(a) for n, a in zip(names, arrs, strict=True)}
        for names, arrs in zip(metas, futs)
    ]


# --------------------------------------------------------------------------
# Launch 1: QKV projections (one shared program, SPMD over sequence shards)
# --------------------------------------------------------------------------

def build_qkv():
    """Per-core: xT [768,512], WqT/WkT/WvT [768,768] (all bf16) ->
    Qt [768,512], Kt [768,512] (transposed) and Vn [512,768], all bf16."""
    nc = bacc.Bacc("TRN2", target_bir_lowering=False, debug=False)
    xT = nc.dram_tensor("xT", [D, SC], BF16, kind="ExternalInput").ap()
    WqT = nc.dram_tensor("WqT", [D, D], BF16, kind="ExternalInput").ap()
    WkT = nc.dram_tensor("WkT", [D, D], BF16, kind="ExternalInput").ap()
    WvT = nc.dram_tensor("WvT", [D, D], BF16, kind="ExternalInput").ap()
    Qt = nc.dram_tensor("Qt", [D, SC], BF16, kind="ExternalOutput").ap()
    Kt = nc.dram_tensor("Kt", [D, SC], BF16, kind="ExternalOutput").ap()
    Vn = nc.dram_tensor("Vn", [SC, D], BF16, kind="ExternalOutput").ap()

    with tile.TileContext(nc) as tc:
        with (
            tc.tile_pool(name="xp", bufs=1) as xp,
            tc.tile_pool(name="wp", bufs=3) as wp,
            tc.tile_pool(name="ps", bufs=4, space="PSUM") as ps,
            tc.tile_pool(name="op", bufs=4) as op,
        ):
            xt_sb = xp.tile([128, NT * SC], BF16)
            for k in range(NT):
                nc.sync.dma_start(
                    xt_sb[:, k * SC:(k + 1) * SC], xT[k * 128:(k + 1) * 128, :])

            def xt(k):
                return xt_sb[:, k * SC:(k + 1) * SC]

            # Q^T / K^T: out tile m = sum_k W^T[k-tile, m-tile]^T @ x^T[k-tile]
            for W_ap, out_ap in ((WqT, Qt), (WkT, Kt)):
                w_sb = wp.tile([128, NT * D], BF16, tag="w")
                for k in range(NT):
                    nc.sync.dma_start(
                        w_sb[:, k * D:(k + 1) * D], W_ap[k * 128:(k + 1) * 128, :])
                for m in range(NT):
                    acc = ps.tile([128, SC], F32, tag="acc")
                    for k in range(NT):
                        nc.tensor.matmul(
                            acc[:],
                            w_sb[:, k * D + m * 128:k * D + (m + 1) * 128],
                            xt(k), start=(k == 0), stop=(k == NT - 1))
                    o = op.tile([128, SC], BF16, tag="o")
                    nc.vector.tensor_copy(o[:], acc[:])
                    nc.sync.dma_start(out_ap[m * 128:(m + 1) * 128, :], o[:])

            # V natural: lhsT = x^T[k, seq-tile], moving = W_v^T[k, :]
            wv_sb = wp.tile([128, NT * D], BF16, tag="w")
            for k in range(NT):
                nc.sync.dma_start(
                    wv_sb[:, k * D:(k + 1) * D], WvT[k * 128:(k + 1) * 128, :])
            for sq in range(SC // 128):
                for n0, n1 in ((0, 512), (512, 768)):
                    acc = ps.tile([128, n1 - n0], F32, tag="acc")
                    for k in range(NT):
                        nc.tensor.matmul(
                            acc[:],
                            xt(k)[:, sq * 128:(sq + 1) * 128],
                            wv_sb[:, k * D + n0:k * D + n1],
                            start=(k == 0), stop=(k == NT - 1))
                    o = op.tile([128, n1 - n0], BF16, tag="o")
                    nc.vector.tensor_copy(o[:], acc[:])
                    nc.sync.dma_start(Vn[sq * 128:(sq + 1) * 128, n0:n1], o[:])
    nc.compile()
    return nc


# --------------------------------------------------------------------------
# Launch 2: attention + W_o (one program variant per core)
# --------------------------------------------------------------------------

def build_attn(core):
    bA, bB = _blocks_for_core(core)
    tA, tB = 2 * bA + 2, 2 * bB + 2   # causal kv-tile counts per block
    SG = 3   # shared-range kv tiles per exp group ([128,1536] = 3 banks)
    BG = 6   # B-only kv tiles per exp group (same psum shape)

    nc = bacc.Bacc("TRN2", target_bir_lowering=False, debug=False)
    Qt = nc.dram_tensor("Qt", [DK, H * SC], BF16, kind="ExternalInput").ap()
    Kt = nc.dram_tensor("Kt", [D, S], BF16, kind="ExternalInput").ap()
    Vaug = nc.dram_tensor("Vaug", [S, H * 65], BF16, kind="ExternalInput").ap()
    WoT = nc.dram_tensor("WoT", [D, D], BF16, kind="ExternalInput").ap()
    E = nc.dram_tensor("E", [H, D], F32R, kind="ExternalInput").ap()
    M0 = nc.dram_tensor("M0", [128, QB], F32, kind="ExternalInput").ap()
    M1 = nc.dram_tensor("M1", [128, QB], F32, kind="ExternalInput").ap()
    yT = nc.dram_tensor("yT", [D, SC], F32, kind="ExternalOutput").ap()

    with tile.TileContext(nc) as tc:
        with (
            tc.tile_pool(name="stat", bufs=1) as stat,
            tc.tile_pool(name="kp", bufs=2) as kp,
            tc.tile_pool(name="vp", bufs=2) as vp,
            tc.tile_pool(name="pp", bufs=3) as pp,
            tc.tile_pool(name="dp", bufs=4) as dp,
        ):
            # Q^T per head at base partition 0: [64, (h, q)]
            qt_sb = stat.tile([64, H * SC], BF16, tag="qt")
            nc.sync.dma_start(qt_sb[:], Qt[:])
            wot_sb = stat.tile([128, NT * D], BF16, tag="wot")
            for g in range(NT):
                nc.sync.dma_start(wot_sb[:, g * D:(g + 1) * D],
                                  WoT[g * 128:(g + 1) * 128, :])
            e_sb = stat.tile([H, D], F32R, tag="e")
            nc.sync.dma_start(e_sb[:], E[:])
            m0_sb = stat.tile([128, QB], F32, tag="m0")
            nc.sync.dma_start(m0_sb[:], M0[:])
            m1_sb = stat.tile([128, QB], F32, tag="m1")
            nc.sync.dma_start(m1_sb[:], M1[:])
            u_sb = stat.tile([128, NT * SC], F32, tag="u")
            d_sb = stat.tile([H, SC], F32, tag="d")

            def q_rhs(h, qo, width):
                return qt_sb[:, h * SC + qo:h * SC + qo + width]

            with (
                tc.tile_pool(name="ps_s", bufs=2, space="PSUM") as ps_s,
                tc.tile_pool(name="ps_u", bufs=1, space="PSUM") as ps_u,
            ):
                for h in range(H):
                    kt_h = kp.tile([64, S], BF16, tag="kt")
                    nc.sync.dma_start(kt_h[:], Kt[h * 64:(h + 1) * 64, :])
                    v_h = vp.tile([128, 32 * 65], BF16, tag="v")
                    nc.sync.dma_start(
                        v_h[:].rearrange("p (t e) -> p t e", e=65),
                        Vaug[:, h * 65:(h + 1) * 65].rearrange(
                            "(t p) e -> p t e", p=128))
                    # A accumulates in psum bank 0 (cols 0:256), B in bank 1
                    # (cols 512:768) — separate banks, separate groups.
                    uacc = ps_u.tile([65, 1024], F32, tag="u")

                    def av(t, p_slice, block):
                        uqo = 0 if block == 0 else 512
                        nc.tensor.matmul(
                            uacc[:, uqo:uqo + QB],
                            v_h[:, t * 65:(t + 1) * 65], p_slice,
                            start=(t == 0),
                            stop=(t == (tA - 1 if block == 0 else tB - 1)),
                            skip_group_check=True)

                    # shared kv range: both blocks at once, N=512
                    for t0 in range(0, tA, SG):
                        tn = min(SG, tA - t0)
                        sc_ps = ps_s.tile([128, SG * SC], F32, tag="s")
                        for i in range(tn):
                            t = t0 + i
                            nc.tensor.matmul(
                                sc_ps[:, i * SC:(i + 1) * SC],
                                kt_h[:, t * 128:(t + 1) * 128],
                                q_rhs(h, 0, SC), start=True, stop=True)
                            if t == tA - 2:
                                nc.vector.tensor_add(
                                    sc_ps[:, i * SC:i * SC + QB],
                                    sc_ps[:, i * SC:i * SC + QB], m0_sb[:])
                            elif t == tA - 1:
                                nc.vector.tensor_add(
                                    sc_ps[:, i * SC:i * SC + QB],
                                    sc_ps[:, i * SC:i * SC + QB], m1_sb[:])
                        p_sb = pp.tile([128, SG * SC], BF16, tag="p")
                        nc.scalar.activation(
                            p_sb[:, :tn * SC], sc_ps[:, :tn * SC], AF.Exp,
                            scale=0.125)
                        for i in range(tn):
                            t = t0 + i
                            av(t, p_sb[:, i * SC:i * SC + QB], 0)
                            av(t, p_sb[:, i * SC + QB:(i + 1) * SC], 1)

                    # B-only kv range: N=256
                    for t0 in range(tA, tB, BG):
                        tn = min(BG, tB - t0)
                        sc_ps = ps_s.tile([128, SG * SC], F32, tag="s")
                        for i in range(tn):
                            t = t0 + i
                            nc.tensor.matmul(
                                sc_ps[:, i * QB:(i + 1) * QB],
                                kt_h[:, t * 128:(t + 1) * 128],
                                q_rhs(h, QB, QB), start=True, stop=True)
                            if t == tB - 2:
                                nc.vector.tensor_add(
                                    sc_ps[:, i * QB:(i + 1) * QB],
                                    sc_ps[:, i * QB:(i + 1) * QB], m0_sb[:])
                            elif t == tB - 1:
                                nc.vector.tensor_add(
                                    sc_ps[:, i * QB:(i + 1) * QB],
                                    sc_ps[:, i * QB:(i + 1) * QB], m1_sb[:])
                        p_sb = pp.tile([128, SG * SC], BF16, tag="p")
                        nc.scalar.activation(
                            p_sb[:, :tn * QB], sc_ps[:, :tn * QB], AF.Exp,
                            scale=0.125)
                        for i in range(tn):
                            t = t0 + i
                            av(t, p_sb[:, i * QB:(i + 1) * QB], 1)

                    g, po = h // 2, 64 * (h % 2)
                    nc.vector.tensor_copy(
                        u_sb[po:po + 64, g * SC:g * SC + QB], uacc[0:64, 0:QB])
                    nc.vector.tensor_copy(
                        u_sb[po:po + 64, g * SC + QB:(g + 1) * SC],
                        uacc[0:64, 512:512 + QB])
                    dtmp = dp.tile([128, SC], F32, tag="dtmp")
                    nc.vector.tensor_copy(dtmp[64:65, 0:QB], uacc[64:65, 0:QB])
                    nc.vector.tensor_copy(
                        dtmp[64:65, QB:SC], uacc[64:65, 512:512 + QB])
                    nc.sync.dma_start(d_sb[h:h + 1, :], dtmp[64:65, :])

            # normalization: R[cdim, q] = 1/denom[head(cdim), q] via selector
            # matmul (partition broadcast), then attn^T = U * R (bf16)
            drec = stat.tile([H, SC], F32R, tag="drec")
            with nc.allow_low_precision(reason="f32r recip for bcast matmul"):
                nc.vector.reciprocal(drec[:], d_sb[:])
            attn_bf = stat.tile([128, NT * SC], BF16, tag="attn")
            with tc.tile_pool(name="ps_r", bufs=2, space="PSUM") as ps_r:
                for g in range(NT):
                    rps = ps_r.tile([128, SC], F32, tag="r")
                    nc.tensor.matmul(
                        rps[:], e_sb[:, g * 128:(g + 1) * 128], drec[:],
                        start=True, stop=True)
                    nc.vector.tensor_mul(
                        attn_bf[:, g * SC:(g + 1) * SC],
                        u_sb[:, g * SC:(g + 1) * SC], rps[:])

            # W_o: y^T[o-tile] = sum_c WoT[c-tile, o-tile]^T @ attn^T[c-tile]
            with (
                tc.tile_pool(name="ps_y", bufs=2, space="PSUM") as ps_y,
                tc.tile_pool(name="yo", bufs=2) as yo,
            ):
                for o in range(NT):
                    yps = ps_y.tile([128, SC], F32, tag="y")
                    for ct in range(NT):
                        nc.tensor.matmul(
                            yps[:],
                            wot_sb[:, ct * D + o * 128:ct * D + (o + 1) * 128],
                            attn_bf[:, ct * SC:(ct + 1) * SC],
                            start=(ct == 0), stop=(ct == NT - 1))
                    yt_sb = yo.tile([128, SC], F32, tag="yt")
                    nc.vector.tensor_copy(yt_sb[:], yps[:])
                    nc.sync.dma_start(yT[o * 128:(o + 1) * 128, :], yt_sb[:])
    nc.compile()
    return nc


# --------------------------------------------------------------------------
# Host-side packing + the public entry point
# --------------------------------------------------------------------------

def _make_masks():
    r = np.arange(128)[:, None]
    j = np.arange(QB)[None, :]
    m0 = np.where(r > j, NEG, 0.0).astype(np.float32)
    m1 = np.where(128 + r > j, NEG, 0.0).astype(np.float32)
    return m0, m1


def _make_E():
    e = np.zeros((H, D), np.float32)
    for h in range(H):
        e[h, h * DK:(h + 1) * DK] = 1.0
    return e


_programs = None


def _get_programs():
    global _programs
    if _programs is None:
        qkv = build_qkv()
        attn = [build_attn(c) for c in range(NC)]
        _programs = (qkv, attn)
    return _programs


def kernel(x, W_q, W_k, W_v, W_o):
    x = np.asarray(x)
    in_dtype = x.dtype
    xs = np.asarray(x, np.float32).reshape(S, D)
    qkv_nc, attn_ncs = _get_programs()

    # ---- launch 1: QKV projections, sequence-sharded ----
    WqT, WkT, WvT = (_bf(np.asarray(w, np.float32).T)
                     for w in (W_q, W_k, W_v))
    in_maps1 = [{
        "xT": _bf(xs[c * SC:(c + 1) * SC].T),
        "WqT": WqT, "WkT": WkT, "WvT": WvT,
    } for c in range(NC)]
    res1 = run_mpmd([qkv_nc] * NC, in_maps1)

    # ---- host gather ----
    Qt_full = np.concatenate([r["Qt"] for r in res1], axis=1)  # [768, 4096]
    Kt_full = np.concatenate([r["Kt"] for r in res1], axis=1)  # [768, 4096]
    V_full = np.concatenate([r["Vn"] for r in res1], axis=0)   # [4096, 768]
    Vaug = np.empty((S, H, 65), ml_dtypes.bfloat16)
    Vaug[:, :, :64] = V_full.reshape(S, H, 64)
    Vaug[:, :, 64] = np.float32(1.0)
    Vaug = Vaug.reshape(S, H * 65)
    Ein = _make_E()
    m0, m1 = _make_masks()

    # ---- launch 2: attention + W_o, query-sharded (zig-zag) ----
    WoT = _bf(np.asarray(W_o, np.float32).T)
    in_maps2 = []
    for c in range(NC):
        bA, bB = _blocks_for_core(c)
        # per-head [64, 512] with that core's two query blocks side by side
        qh = np.empty((DK, H * SC), ml_dtypes.bfloat16)
        for h in range(H):
            qh[:, h * SC:h * SC + QB] = \
                Qt_full[h * DK:(h + 1) * DK, bA * QB:(bA + 1) * QB]
            qh[:, h * SC + QB:(h + 1) * SC] = \
                Qt_full[h * DK:(h + 1) * DK, bB * QB:(bB + 1) * QB]
        in_maps2.append({
            "Qt": qh, "Kt": Kt_full, "Vaug": Vaug, "WoT": WoT,
            "E": Ein, "M0": m0, "M1": m1,
        })
    res2 = run_mpmd(attn_ncs, in_maps2)

    # ---- host scatter ----
    y = np.empty((S, D), np.float32)
    for c in range(NC):
        bA, bB = _blocks_for_core(c)
        yc = res2[c]["yT"].T  # [512, 768]
        y[bA * QB:(bA + 1) * QB] = yc[:QB]
        y[bB * QB:(bB + 1) * QB] = yc[QB:]
    return y.reshape(B, S, D).astype(in_dtype, copy=False)


# revision 6
# speedup vs baseline: 1.1108x; 1.1108x over previous
"""Trainium2 Bass kernel for causal multi-head self-attention.

nn.Module: y = MHSA(x) with D=768, H=12 heads, d_k=64, S=4096, causal mask,
torch-Linear convention (y = x @ W.T, no bias).

Distribution over the 8 NeuronCores (no collectives — host-side gather
between two device launches):

  Launch 1 (SPMD, all cores run the same program): QKV projections,
  sequence-sharded. Core c projects x rows [512c, 512c+512) against all of
  W_q/W_k/W_v, emitting Q^T and K^T (head-dim-major) and V (natural), all
  bf16. The host concatenates the shards (pure gather).

  Launch 2 (MPMD, one program variant per core): attention + W_o,
  query-sharded with zig-zag causal load balancing: core c owns the two
  256-row query blocks (c, 15-c), so every core does an equal amount of
  causal work. Scores are computed transposed (scores^T[kv, q], K-tile
  stationary / Q^T moving) so the softmax numerator feeds the P^T @ V
  matmul with no PE transposes. Softmax skips max-subtraction (scores are
  ~N(0,1); exp cannot overflow in fp32) and gets denominators for free via
  a ones-column appended to V. The strict-upper causal mask is applied
  additively (-1e9) to the two diagonal kv-tiles per block only. The
  per-column denominators are broadcast across partitions with a tiny
  selector matmul on the PE, and the normalized attention output (already
  transposed = exactly the W_o contraction layout) goes through W_o.
  Core c returns y rows for its two blocks; the host scatters them back.

All matmuls are bf16 with fp32 PSUM accumulation.
"""

import numpy as np
import ml_dtypes
import jax

import concourse.bass as bass
import concourse.tile as tile
import concourse.mybir as mybir
from concourse import bacc, bass2jax

BF16 = mybir.dt.bfloat16
F32 = mybir.dt.float32
F32R = mybir.dt.float32r
AF = mybir.ActivationFunctionType

B = 1
D = 768          # d_model
S = 4096         # sequence length
H = 12           # heads
DK = 64          # head dim
NC = 8           # NeuronCores
NB = 16          # 256-row query blocks
QB = S // NB     # 256
SC = S // NC     # 512 rows per core
NT = D // 128    # 6
NEG = -1e9

_bf = lambda a: np.ascontiguousarray(a).astype(ml_dtypes.bfloat16)


def _blocks_for_core(c):
    return (c, NB - 1 - c)


# --------------------------------------------------------------------------
# MPMD runner: run a (possibly different) bass program on each NeuronCore
# concurrently via the bass_exec custom-call machinery.
# --------------------------------------------------------------------------

def _io_names(nc):
    in_names, out_names, out_avals = [], [], []
    pname = nc.partition_id_tensor.name if nc.partition_id_tensor else None
    for alloc in nc.m.functions[0].allocations:
        if not isinstance(alloc, mybir.MemoryLocationSet):
            continue
        name = alloc.memorylocations[0].name
        if alloc.kind == "ExternalInput":
            if name != pname:
                in_names.append(name)
        elif alloc.kind == "ExternalOutput":
            out_names.append(name)
            out_avals.append(
                jax.core.ShapedArray(
                    tuple(alloc.tensor_shape), mybir.dt.np(alloc.dtype)))
    return in_names, out_names, out_avals, pname


_jit_cache = {}


def run_mpmd(ncs, in_maps):
    """ncs: one compiled Bacc program per core (entries may repeat);
    in_maps: per-core dict name->np.ndarray. Returns per-core output dicts."""
    bass2jax.install_neuronx_cc_hook()
    devices = jax.devices()[: len(ncs)]
    futs, metas = [], []
    for core_id, (nc, in_map, dev) in enumerate(
            zip(ncs, in_maps, devices, strict=True)):
        in_names, out_names, out_avals, pname = _io_names(nc)
        key = (id(nc), core_id)
        if key not in _jit_cache:
            all_names = tuple(in_names + out_names + ([pname] if pname else []))

            def _body(*args, _nc=nc, _avals=tuple(out_avals),
                      _names=all_names, _onames=tuple(out_names)):
                return tuple(bass2jax._bass_exec_p.bind(
                    *args, out_avals=_avals, in_names=_names,
                    out_names=_onames, lowering_input_output_aliases=(),
                    sim_require_finite=True, sim_require_nnan=True, nc=_nc))

            n_params = len(in_names)
            donate = tuple(range(n_params, n_params + len(out_avals)))
            _jit_cache[key] = jax.jit(
                _body, donate_argnums=donate, keep_unused=True)
        fn = _jit_cache[key]
        dev_args = [jax.device_put(np.asarray(in_map[n]), dev)
                    for n in in_names]
        dev_zeros = [jax.device_put(np.zeros(a.shape, a.dtype), dev)
                     for a in out_avals]
        extra = ([jax.device_put(np.array([[core_id]], np.uint32), dev)]
                 if pname else [])
        futs.append(fn(*dev_args, *dev_zeros, *extra))
        metas.append(out_names)
    return [
        {n: np.asarray(a) for n, a in zip(names, arrs, strict=True)}
        for names, arrs in zip(metas, futs)
    ]


# --------------------------------------------------------------------------
# Launch 1: QKV projections (one shared program, SPMD over sequence shards)
# --------------------------------------------------------------------------

def build_qkv():
    """Per-core: xT [768,512], WqT/WkT/WvT [768,768] (all bf16) ->
    Qt [768,512], Kt [768,512] (transposed) and Vn [512,768], all bf16."""
    nc = bacc.Bacc("TRN2", target_bir_lowering=False, debug=False)
    xT = nc.dram_tensor("xT", [D, SC], BF16, kind="ExternalInput").ap()
    WqT = nc.dram_tensor("WqT", [D, D], BF16, kind="ExternalInput").ap()
    WkT = nc.dram_tensor("WkT", [D, D], BF16, kind="ExternalInput").ap()
    WvT = nc.dram_tensor("WvT", [D, D], BF16, kind="ExternalInput").ap()
    Qt = nc.dram_tensor("Qt", [D, SC], BF16, kind="ExternalOutput").ap()
    Kt = nc.dram_tensor("Kt", [D, SC], BF16, kind="ExternalOutput").ap()
    Vn = nc.dram_tensor("Vn", [SC, D], BF16, kind="ExternalOutput").ap()

    with tile.TileContext(nc) as tc:
        with (
            tc.tile_pool(name="xp", bufs=1) as xp,
            tc.tile_pool(name="wp", bufs=3) as wp,
            tc.tile_pool(name="ps", bufs=4, space="PSUM") as ps,
            tc.tile_pool(name="op", bufs=4) as op,
        ):
            xt_sb = xp.tile([128, NT * SC], BF16)
            for k in range(NT):
                nc.sync.dma_start(
                    xt_sb[:, k * SC:(k + 1) * SC], xT[k * 128:(k + 1) * 128, :])

            def xt(k):
                return xt_sb[:, k * SC:(k + 1) * SC]

            # Q^T / K^T: out tile m = sum_k W^T[k-tile, m-tile]^T @ x^T[k-tile]
            for W_ap, out_ap in ((WqT, Qt), (WkT, Kt)):
                w_sb = wp.tile([128, NT * D], BF16, tag="w")
                for k in range(NT):
                    nc.sync.dma_start(
                        w_sb[:, k * D:(k + 1) * D], W_ap[k * 128:(k + 1) * 128, :])
                for m in range(NT):
                    acc = ps.tile([128, SC], F32, tag="acc")
                    for k in range(NT):
                        nc.tensor.matmul(
                            acc[:],
                            w_sb[:, k * D + m * 128:k * D + (m + 1) * 128],
                            xt(k), start=(k == 0), stop=(k == NT - 1))
                    o = op.tile([128, SC], BF16, tag="o")
                    nc.vector.tensor_copy(o[:], acc[:])
                    nc.sync.dma_start(out_ap[m * 128:(m + 1) * 128, :], o[:])

            # V natural: lhsT = x^T[k, seq-tile], moving = W_v^T[k, :]
            wv_sb = wp.tile([128, NT * D], BF16, tag="w")
            for k in range(NT):
                nc.sync.dma_start(
                    wv_sb[:, k * D:(k + 1) * D], WvT[k * 128:(k + 1) * 128, :])
            for sq in range(SC // 128):
                for n0, n1 in ((0, 512), (512, 768)):
                    acc = ps.tile([128, n1 - n0], F32, tag="acc")
                    for k in range(NT):
                        nc.tensor.matmul(
                            acc[:],
                            xt(k)[:, sq * 128:(sq + 1) * 128],
                            wv_sb[:, k * D + n0:k * D + n1],
                            start=(k == 0), stop=(k == NT - 1))
                    o = op.tile([128, n1 - n0], BF16, tag="o")
                    nc.vector.tensor_copy(o[:], acc[:])
                    nc.sync.dma_start(Vn[sq * 128:(sq + 1) * 128, n0:n1], o[:])
    nc.compile()
    return nc


# --------------------------------------------------------------------------
# Launch 2: attention + W_o (one program variant per core)
# --------------------------------------------------------------------------

def build_attn(core):
    bA, bB = _blocks_for_core(core)
    tA, tB = 2 * bA + 2, 2 * bB + 2   # causal kv-tile counts per block
    SG = 3   # shared-range kv tiles per exp group ([128,1536] = 3 banks)
    BG = 6   # B-only kv tiles per exp group (same psum shape)

    nc = bacc.Bacc("TRN2", target_bir_lowering=False, debug=False)
    Qt = nc.dram_tensor("Qt", [DK, H * SC], BF16, kind="ExternalInput").ap()
    Kt = nc.dram_tensor("Kt", [D, S], BF16, kind="ExternalInput").ap()
    Vaug = nc.dram_tensor("Vaug", [S, H * 65], BF16, kind="ExternalInput").ap()
    WoT = nc.dram_tensor("WoT", [D, D], BF16, kind="ExternalInput").ap()
    Ident = nc.dram_tensor("Ident", [128, 128], BF16, kind="ExternalInput").ap()
    M0 = nc.dram_tensor("M0", [128, QB], F32, kind="ExternalInput").ap()
    M1 = nc.dram_tensor("M1", [128, QB], F32, kind="ExternalInput").ap()
    yT = nc.dram_tensor("yT", [D, SC], F32, kind="ExternalOutput").ap()

    with tile.TileContext(nc) as tc:
        with (
            tc.tile_pool(name="stat", bufs=1) as stat,
            tc.tile_pool(name="kp", bufs=2) as kp,
            tc.tile_pool(name="vp", bufs=2) as vp,
            tc.tile_pool(name="pp", bufs=3) as pp,
            tc.tile_pool(name="dp", bufs=4) as dp,
        ):
            # Q^T per head at base partition 0: [64, (h, q)]
            qt_sb = stat.tile([64, H * SC], BF16, tag="qt")
            nc.sync.dma_start(qt_sb[:], Qt[:])
            wot_sb = stat.tile([128, NT * D], BF16, tag="wot")
            for g in range(NT):
                nc.sync.dma_start(wot_sb[:, g * D:(g + 1) * D],
                                  WoT[g * 128:(g + 1) * 128, :])
            id_sb = stat.tile([128, 128], BF16, tag="ident")
            nc.sync.dma_start(id_sb[:], Ident[:])
            m0_sb = stat.tile([128, QB], F32, tag="m0")
            nc.sync.dma_start(m0_sb[:], M0[:])
            m1_sb = stat.tile([128, QB], F32, tag="m1")
            nc.sync.dma_start(m1_sb[:], M1[:])
            # normalized attention output, natural layout:
            # [128 q, (qsub, h*64+d)] bf16
            attn_nat = stat.tile([128, 4 * D], BF16, tag="attn_nat")

            def q_rhs(h, qo, width):
                return qt_sb[:, h * SC + qo:h * SC + qo + width]

            with (
                tc.tile_pool(name="ps_s", bufs=2, space="PSUM") as ps_s,
                tc.tile_pool(name="ps_u", bufs=1, space="PSUM") as ps_u,
            ):
                for h in range(H):
                    kt_h = kp.tile([64, S], BF16, tag="kt")
                    nc.sync.dma_start(kt_h[:], Kt[h * 64:(h + 1) * 64, :])
                    v_h = vp.tile([128, 32 * 65], BF16, tag="v")
                    nc.sync.dma_start(
                        v_h[:].rearrange("p (t e) -> p t e", e=65),
                        Vaug[:, h * 65:(h + 1) * 65].rearrange(
                            "(t p) e -> p t e", p=128))
                    # natural-layout AV accumulators, one per 128-q sub-tile:
                    # bank0 = block A (cols 0:65 | 65:130),
                    # bank1 = block B (cols 512:577 | 577:642).
                    # Only the first mm in each bank uses start=True: it marks
                    # the whole 2KB bank pending-zero; first write per byte
                    # then overwrites, later writes accumulate.
                    unat = ps_u.tile([128, 1024], F32, tag="u")

                    def av(t, p_slice, block, sub):
                        uqo = (0 if block == 0 else 512) + sub * 65
                        nc.tensor.matmul(
                            unat[:, uqo:uqo + 65],
                            p_slice,
                            v_h[:, t * 65:(t + 1) * 65],
                            start=(t == 0 and sub == 0),
                            stop=(t == (tA - 1 if block == 0 else tB - 1)
                                  and sub == 1),
                            skip_group_check=True)

                    # shared kv range: both blocks at once, N=512
                    for t0 in range(0, tA, SG):
                        tn = min(SG, tA - t0)
                        sc_ps = ps_s.tile([128, SG * SC], F32, tag="s")
                        for i in range(tn):
                            t = t0 + i
                            nc.tensor.matmul(
                                sc_ps[:, i * SC:(i + 1) * SC],
                                kt_h[:, t * 128:(t + 1) * 128],
                                q_rhs(h, 0, SC), start=True, stop=True)
                            if t == tA - 2:
                                nc.vector.tensor_add(
                                    sc_ps[:, i * SC:i * SC + QB],
                                    sc_ps[:, i * SC:i * SC + QB], m0_sb[:])
                            elif t == tA - 1:
                                nc.vector.tensor_add(
                                    sc_ps[:, i * SC:i * SC + QB],
                                    sc_ps[:, i * SC:i * SC + QB], m1_sb[:])
                        p_sb = pp.tile([128, SG * SC], BF16, tag="p")
                        nc.scalar.activation(
                            p_sb[:, :tn * SC], sc_ps[:, :tn * SC], AF.Exp,
                            scale=0.125)
                        for i in range(tn):
                            t = t0 + i
                            for sub in (0, 1):
                                av(t, p_sb[:, i * SC + sub * 128:
                                           i * SC + (sub + 1) * 128], 0, sub)
                                av(t, p_sb[:, i * SC + QB + sub * 128:
                                           i * SC + QB + (sub + 1) * 128], 1, sub)

                    # B-only kv range: N=256
                    for t0 in range(tA, tB, BG):
                        tn = min(BG, tB - t0)
                        sc_ps = ps_s.tile([128, SG * SC], F32, tag="s")
                        for i in range(tn):
                            t = t0 + i
                            nc.tensor.matmul(
                                sc_ps[:, i * QB:(i + 1) * QB],
                                kt_h[:, t * 128:(t + 1) * 128],
                                q_rhs(h, QB, QB), start=True, stop=True)
                            if t == tB - 2:
                                nc.vector.tensor_add(
                                    sc_ps[:, i * QB:(i + 1) * QB],
                                    sc_ps[:, i * QB:(i + 1) * QB], m0_sb[:])
                            elif t == tB - 1:
                                nc.vector.tensor_add(
                                    sc_ps[:, i * QB:(i + 1) * QB],
                                    sc_ps[:, i * QB:(i + 1) * QB], m1_sb[:])
                        p_sb = pp.tile([128, SG * SC], BF16, tag="p")
                        nc.scalar.activation(
                            p_sb[:, :tn * QB], sc_ps[:, :tn * QB], AF.Exp,
                            scale=0.125)
                        for i in range(tn):
                            t = t0 + i
                            for sub in (0, 1):
                                av(t, p_sb[:, i * QB + sub * 128:
                                           i * QB + (sub + 1) * 128], 1, sub)

                    # normalize: denominators are per-partition scalars now
                    for block, sub in ((0, 0), (0, 1), (1, 0), (1, 1)):
                        uqo = (0 if block == 0 else 512) + sub * 65
                        qsub = block * 2 + sub
                        r = dp.tile([128, 1], F32, tag="recip")
                        nc.vector.reciprocal(r[:], unat[:, uqo + 64:uqo + 65])
                        nc.vector.tensor_scalar_mul(
                            attn_nat[:, qsub * D + h * DK:
                                     qsub * D + (h + 1) * DK],
                            unat[:, uqo:uqo + 64], r[:])

            # transpose attn_nat -> attn_bf^T [c-part, q-free] for W_o
            attn_bf = stat.tile([128, NT * SC], BF16, tag="attn")
            with (
                tc.tile_pool(name="ps_t", bufs=4, space="PSUM") as ps_t,
            ):
                for qsub in range(4):
                    for g in range(NT):
                        tps = ps_t.tile([128, 128], BF16, tag="t")
                        nc.tensor.transpose(
                            tps[:],
                            attn_nat[:, qsub * D + g * 128:
                                     qsub * D + (g + 1) * 128],
                            id_sb[:])
                        nc.vector.tensor_copy(
                            attn_bf[:, g * SC + qsub * 128:
                                    g * SC + (qsub + 1) * 128], tps[:])

            # W_o: y^T[o-tile] = sum_c WoT[c-tile, o-tile]^T @ attn^T[c-tile]
            with (
                tc.tile_pool(name="ps_y", bufs=2, space="PSUM") as ps_y,
                tc.tile_pool(name="yo", bufs=2) as yo,
            ):
                for o in range(NT):
                    yps = ps_y.tile([128, SC], F32, tag="y")
                    for ct in range(NT):
                        nc.tensor.matmul(
                            yps[:],
                            wot_sb[:, ct * D + o * 128:ct * D + (o + 1) * 128],
                            attn_bf[:, ct * SC:(ct + 1) * SC],
                            start=(ct == 0), stop=(ct == NT - 1))
                    yt_sb = yo.tile([128, SC], F32, tag="yt")
                    nc.vector.tensor_copy(yt_sb[:], yps[:])
                    nc.sync.dma_start(yT[o * 128:(o + 1) * 128, :], yt_sb[:])
    nc.compile()
    return nc


# --------------------------------------------------------------------------
# Host-side packing + the public entry point
# --------------------------------------------------------------------------

def _make_masks():
    r = np.arange(128)[:, None]
    j = np.arange(QB)[None, :]
    m0 = np.where(r > j, NEG, 0.0).astype(np.float32)
    m1 = np.where(128 + r > j, NEG, 0.0).astype(np.float32)
    return m0, m1


def _make_ident():
    return np.eye(128, dtype=ml_dtypes.bfloat16)


_programs = None


def _get_programs():
    global _programs
    if _programs is None:
        qkv = build_qkv()
        attn = [build_attn(c) for c in range(NC)]
        _programs = (qkv, attn)
    return _programs


def kernel(x, W_q, W_k, W_v, W_o):
    x = np.asarray(x)
    in_dtype = x.dtype
    xs = np.asarray(x, np.float32).reshape(S, D)
    qkv_nc, attn_ncs = _get_programs()

    # ---- launch 1: QKV projections, sequence-sharded ----
    WqT, WkT, WvT = (_bf(np.asarray(w, np.float32).T)
                     for w in (W_q, W_k, W_v))
    in_maps1 = [{
        "xT": _bf(xs[c * SC:(c + 1) * SC].T),
        "WqT": WqT, "WkT": WkT, "WvT": WvT,
    } for c in range(NC)]
    res1 = run_mpmd([qkv_nc] * NC, in_maps1)

    # ---- host gather ----
    Qt_full = np.concatenate([r["Qt"] for r in res1], axis=1)  # [768, 4096]
    Kt_full = np.concatenate([r["Kt"] for r in res1], axis=1)  # [768, 4096]
    V_full = np.concatenate([r["Vn"] for r in res1], axis=0)   # [4096, 768]
    Vaug = np.empty((S, H, 65), ml_dtypes.bfloat16)
    Vaug[:, :, :64] = V_full.reshape(S, H, 64)
    Vaug[:, :, 64] = np.float32(1.0)
    Vaug = Vaug.reshape(S, H * 65)
    ident = _make_ident()
    m0, m1 = _make_masks()

    # ---- launch 2: attention + W_o, query-sharded (zig-zag) ----
    WoT = _bf(np.asarray(W_o, np.float32).T)
    in_maps2 = []
    for c in range(NC):
        bA, bB = _blocks_for_core(c)
        # per-head [64, 512] with that core's two query blocks side by side
        qh = np.empty((DK, H * SC), ml_dtypes.bfloat16)
        for h in range(H):
            qh[:, h * SC:h * SC + QB] = \
                Qt_full[h * DK:(h + 1) * DK, bA * QB:(bA + 1) * QB]
            qh[:, h * SC + QB:(h + 1) * SC] = \
                Qt_full[h * DK:(h + 1) * DK, bB * QB:(bB + 1) * QB]
        in_maps2.append({
            "Qt": qh, "Kt": Kt_full, "Vaug": Vaug, "WoT": WoT,
            "Ident": ident, "M0": m0, "M1": m1,
        })
    res2 = run_mpmd(attn_ncs, in_maps2)

    # ---- host scatter ----
    y = np.empty((S, D), np.float32)
    for c in range(NC):
        bA, bB = _blocks_for_core(c)
        yc = res2[c]["yT"].T  # [512, 768]
        y[bA * QB:(bA + 1) * QB] = yc[:QB]
        y[bB * QB:(bB + 1) * QB] = yc[QB:]
    return y.reshape(B, S, D).astype(in_dtype, copy=False)
